# revision 6
# baseline (speedup 1.0000x reference)
"""Multi-head attention (B=16, T=1024, D=768, H=12) on 8 TRN2 NeuronCores.

Strategy: pure data parallelism over the batch — each core computes full MHA
for 2 batch elements. No collectives.

v2 changes over the 381us baseline:
  - q/k projections run as fp8e4 DoubleRow matmuls (weights pre-scaled x32 on
    host, bias folded as 32*b, exp scale divided by 32*32). v/O/y stay bf16.
  - All projection work (v-proj, q/k-proj, y-proj) is emitted through a drip
    queue that releases ~1 unit per attention sc-iteration, so the exp-bound
    middle absorbs projection matmuls in its PE slack instead of running them
    as dense PE-only phases.
  - Lean HAM warmup (N=128 matmuls instead of N=512).
  - ps_o gets 3 PSUM banks (th-boundary overlap), drip pool drops to 1.
  - l (softmax denominator) rows leave PSUM by direct DMA (no DVE copy).
  - y(last batch) runs on the ps_s pool banks after attention finishes.
"""

import os
from collections import deque
from contextlib import ExitStack

import numpy as np
import ml_dtypes

import concourse.bacc as bacc
import concourse.bass as bass
import concourse.mybir as mybir
import concourse.tile as tile
from concourse.bass_utils import run_bass_kernel_spmd

BF16 = ml_dtypes.bfloat16
F8E4 = ml_dtypes.float8_e4m3

# Full problem dims
B, T_FULL, D_FULL, H, HS = 16, 1024, 768, 12, 64
N_CORES = 8
NB = B // N_CORES  # batch elements per core
W8SCALE = 32.0     # host pre-scale on Wq/Wk before fp8 cast


def build_mha_nc(nb, t, d, npair, trn_type="TRN2", variant="fp8qk"):
    P = 128
    KC = d // P              # contraction chunks over model dim
    KC2 = KC // 2            # DoubleRow chunk pairs
    SC = t // P              # s (key position) chunks
    NTH = max(1, t // 512)   # output-column groups for S/O matmuls
    TW = t // NTH            # width of each group (<= 512)
    TC = t // P              # t row chunks for v/y
    D2 = d // 2              # y-proj free-dim split (<= 512 fp32 psum)
    dpair = 2 * HS           # 128
    fp8qk = "fp8qk" in variant
    scale = 1.0 / np.sqrt(HS)
    if fp8qk:
        scale /= W8SCALE * W8SCALE

    f32 = mybir.dt.float32
    bf16 = mybir.dt.bfloat16
    f8e4 = mybir.dt.float8e4
    AF = mybir.ActivationFunctionType
    DR = mybir.MatmulPerfMode.DoubleRow

    nc = bacc.Bacc(trn_type, target_bir_lowering=False, debug=False)

    xt_d = nc.dram_tensor("xt", [nb, d, t], bf16, kind="ExternalInput").ap()
    if fp8qk:
        xt8_d = nc.dram_tensor("xt8", [nb, d, t], f8e4, kind="ExternalInput").ap()
        wq_d = nc.dram_tensor("wq", [P, npair, KC2, 2, dpair], f8e4, kind="ExternalInput").ap()
        wk_d = nc.dram_tensor("wk", [P, npair, KC2, 2, dpair], f8e4, kind="ExternalInput").ap()
    else:
        wq_d = nc.dram_tensor("wq", [P, npair, KC, dpair], bf16, kind="ExternalInput").ap()
        wk_d = nc.dram_tensor("wk", [P, npair, KC, dpair], bf16, kind="ExternalInput").ap()
    wv_d = nc.dram_tensor("wv", [P, KC, npair * dpair], bf16, kind="ExternalInput").ap()
    wp_d = nc.dram_tensor("wp", [P, KC, d], bf16, kind="ExternalInput").ap()
    bqk_d = nc.dram_tensor("bqk", [P, npair, 2], f32, kind="ExternalInput").ap()
    bv_d = nc.dram_tensor("bv", [P, npair, dpair], bf16, kind="ExternalInput").ap()
    bp_d = nc.dram_tensor("bp", [P, d], f32, kind="ExternalInput").ap()
    y_d = nc.dram_tensor("y", [nb, t, d], f32, kind="ExternalOutput").ap()

    with TileOrExit(nc) as (tc, ctx):
        # ---- persistent weights ----
        p_w = ctx.enter_context(tc.tile_pool(name="p_w", bufs=1))
        if fp8qk:
            wq_sb = p_w.tile([P, npair, KC2, 2, dpair], f8e4, tag="wq", name="wq_sb")
            wk_sb = p_w.tile([P, npair, KC2, 2, dpair], f8e4, tag="wk", name="wk_sb")
        else:
            wq_sb = p_w.tile([P, npair, KC, dpair], bf16, tag="wq", name="wq_sb")
            wk_sb = p_w.tile([P, npair, KC, dpair], bf16, tag="wk", name="wk_sb")
        wv_sb = p_w.tile([P, KC, npair * dpair], bf16, tag="wv", name="wv_sb")
        wp_sb = p_w.tile([P, KC, d], bf16, tag="wp", name="wp_sb")
        bqk_sb = p_w.tile([P, npair, 2], f32, tag="bqk", name="bqk_sb")
        bv_sb = p_w.tile([P, npair, dpair], bf16, tag="bv", name="bv_sb")
        bp_sb = p_w.tile([P, d], f32, tag="bp", name="bp_sb")
        for c in range(KC):
            nc.gpsimd.dma_start(wv_sb[:, c], wv_d[:, c])
        nc.gpsimd.dma_start(bv_sb[:], bv_d)
        for pr in range(npair):
            nc.gpsimd.dma_start(wq_sb[:, pr], wq_d[:, pr])
            nc.gpsimd.dma_start(wk_sb[:, pr], wk_d[:, pr])
        nc.gpsimd.dma_start(bqk_sb[:], bqk_d)
        nc.gpsimd.dma_start(wp_sb[:], wp_d)
        nc.gpsimd.dma_start(bp_sb[:], bp_d)

        # ---- pools ----
        p_xt = ctx.enter_context(tc.tile_pool(name="p_xt", bufs=2))
        if fp8qk:
            p_xt8 = ctx.enter_context(tc.tile_pool(name="p_xt8", bufs=2))
        p_vall = ctx.enter_context(tc.tile_pool(name="p_vall", bufs=2))
        p_qk = ctx.enter_context(tc.tile_pool(name="p_qk", bufs=3))
        p_es = ctx.enter_context(tc.tile_pool(name="p_es", bufs=3))
        p_oall = ctx.enter_context(tc.tile_pool(name="p_oall", bufs=1))
        p_norm = ctx.enter_context(tc.tile_pool(name="p_norm", bufs=2))
        p_y = ctx.enter_context(tc.tile_pool(name="p_y", bufs=2))
        # PSUM: ps_s 2x2 banks, ps_o 3x1, ps_m 1x1 => 8 banks
        ps_s = ctx.enter_context(tc.tile_pool(name="ps_s", bufs=2, space="PSUM"))
        ps_o = ctx.enter_context(tc.tile_pool(name="ps_o", bufs=3, space="PSUM"))
        ps_m = ctx.enter_context(tc.tile_pool(name="ps_m", bufs=1, space="PSUM"))

        xts = [None] * nb
        xt8s = [None] * nb
        v_alls = [None] * nb
        o_allTs = [None] * nb
        qkTs = {}   # (b, pr) -> (qT, kT)
        y_sbs = {}  # (b, tci) -> y_sb

        def load_xt(b):
            xt = p_xt.tile([P, KC, t], bf16, tag="xt", name="xt_sb")
            xt_src = xt_d[b].rearrange("(c p) t -> p c t", p=P)
            for c in range(KC):
                nc.sync.dma_start(xt[:, c], xt_src[:, c])
            xts[b] = xt
            if fp8qk:
                xt8 = p_xt8.tile([P, KC, t], f8e4, tag="xt8", name="xt8_sb")
                xt8_src = xt8_d[b].rearrange("(c p) t -> p c t", p=P)
                for c in range(KC):
                    nc.sync.dma_start(xt8[:, c], xt8_src[:, c])
                xt8s[b] = xt8

        def prep_vall(b):
            v_alls[b] = p_vall.tile([P, SC, npair, 130], bf16, tag="vall", name="v_all")
            ones_view = v_alls[b].rearrange("p s r (h x) -> p s r h x", h=2)
            nc.gpsimd.memset(ones_view[:, :, :, :, 64:65], 1.0)
            o_allTs[b] = p_oall.tile([P, npair, t], bf16, tag=f"oall{b}", name="o_allT")

        # ---------- drip units ----------
        def v_group_unit(b, tci, g, gn=3):
            def run():
                xt = xts[b]
                psv = ps_m.tile([P, TW], f32, tag="m", name="psv")
                for c in range(KC):
                    nc.tensor.matmul(
                        psv[:, : gn * dpair],
                        lhsT=xt[:, c, tci * P : (tci + 1) * P],
                        rhs=wv_sb[:, c, 3 * g * dpair : (3 * g + gn) * dpair],
                        start=(c == 0),
                        stop=(c == KC - 1),
                    )
                glo = 3 * g
                dst = v_alls[b][:, tci, glo : glo + gn, :].rearrange(
                    "p r (h x) -> p r h x", h=2
                )[:, :, :, 0:64]
                src = psv[:, : gn * dpair].rearrange("p (r h e) -> p r h e", r=gn, h=2)
                bias = bv_sb[:, glo : glo + gn, :].rearrange("p r (h e) -> p r h e", h=2)
                nc.vector.tensor_add(out=dst, in0=src, in1=bias)
            return run

        def qk_unit(b, pr, which, th):
            def run():
                if (b, pr) not in qkTs:
                    qkTs[(b, pr)] = (
                        p_qk.tile([P, t], bf16, tag="qT", name="qT"),
                        p_qk.tile([P, t], bf16, tag="kT", name="kT"),
                    )
                w_sb = wq_sb if which == 0 else wk_sb
                dstT = qkTs[(b, pr)][which]
                psq = ps_m.tile([P, TW], f32, tag="m", name="psq")
                if fp8qk:
                    xt8 = xt8s[b]
                    for i in range(KC2):
                        nc.tensor.matmul(
                            psq[:],
                            lhsT=w_sb[:, pr, i, :, :],
                            rhs=xt8[:, 2 * i : 2 * i + 2, th * TW : (th + 1) * TW],
                            start=(i == 0),
                            stop=(i == KC2 - 1),
                            perf_mode=DR,
                        )
                else:
                    xt = xts[b]
                    for c in range(KC):
                        nc.tensor.matmul(
                            psq[:],
                            lhsT=w_sb[:, pr, c, :],
                            rhs=xt[:, c, th * TW : (th + 1) * TW],
                            start=(c == 0),
                            stop=(c == KC - 1),
                        )
                nc.vector.tensor_scalar_add(
                    out=dstT[:, th * TW : (th + 1) * TW],
                    in0=psq[:],
                    scalar1=bqk_sb[:, pr, which : which + 1],
                )
            return run

        def y_unit(b, tci, j):
            def run():
                if (b, tci) not in y_sbs:
                    y_sbs[(b, tci)] = p_y.tile([P, d], f32, tag="y", name="y_sb")
                y_sb = y_sbs[(b, tci)]
                psy = ps_m.tile([P, TW], f32, tag="m", name="psy")
                for c in range(KC):
                    nc.tensor.matmul(
                        psy[:, 0:D2],
                        lhsT=o_allTs[b][:, c, tci * P : (tci + 1) * P],
                        rhs=wp_sb[:, c, j * D2 : (j + 1) * D2],
                        start=(c == 0),
                        stop=(c == KC - 1),
                    )
                nc.vector.tensor_add(
                    out=y_sb[:, j * D2 : (j + 1) * D2],
                    in0=psy[:, 0:D2],
                    in1=bp_sb[:, j * D2 : (j + 1) * D2],
                )
                if j == 1:
                    nc.sync.dma_start(out=y_d[b, tci * P : (tci + 1) * P, :], in_=y_sb[:])
            return run

        def y_unit_tail(b, tci):
            def run():
                psy = ps_s.tile([P, 2, TW], f32, tag="s", name="ps_s")
                for c in range(KC):
                    for j in range(2):
                        nc.tensor.matmul(
                            psy[:, j, 0:D2],
                            lhsT=o_allTs[b][:, c, tci * P : (tci + 1) * P],
                            rhs=wp_sb[:, c, j * D2 : (j + 1) * D2],
                            start=(c == 0),
                            stop=(c == KC - 1),
                        )
                y_sb = p_y.tile([P, d], f32, tag="y", name="y_sb")
                for j in range(2):
                    nc.vector.tensor_add(
                        out=y_sb[:, j * D2 : (j + 1) * D2],
                        in0=psy[:, j, 0:D2],
                        in1=bp_sb[:, j * D2 : (j + 1) * D2],
                    )
                nc.sync.dma_start(out=y_d[b, tci * P : (tci + 1) * P, :], in_=y_sb[:])
            return run

        drip = deque()

        def pump():
            if drip:
                drip.popleft()()

        # ---------- attention middle for one (b, pair) ----------
        def attention_pair(b, pr):
            qT, kT = qkTs[(b, pr)]
            v_all = v_alls[b]
            o_allT = o_allTs[b]
            for th in range(NTH):
                esq = [None] * SC
                psos = [ps_o.tile([65, TW], f32, tag="o", name="pso") for _ in range(2)]
                for sc in range(SC + 2):
                    if sc < SC:
                        esq[sc] = p_es.tile([P, 2, TW], bf16, tag="es", name="es")
                        ps = ps_s.tile([P, 2, TW], f32, tag="s", name="ps_s")
                        nc.tensor.matmul(
                            ps[:, 0, :],
                            lhsT=kT[0:64, sc * P : (sc + 1) * P],
                            rhs=qT[0:64, th * TW : (th + 1) * TW],
                            start=True,
                            stop=True,
                        )
                        nc.tensor.matmul(
                            ps[:, 1, :],
                            lhsT=kT[64:128, sc * P : (sc + 1) * P],
                            rhs=qT[64:128, th * TW : (th + 1) * TW],
                            start=True,
                            stop=True,
                            tile_position=(64, 0),
                        )
                        nc.scalar.activation(
                            out=esq[sc][:], in_=ps[:], func=AF.Exp, scale=scale
                        )
                    if sc >= 2:
                        so = sc - 2
                        for h in range(2):
                            nc.tensor.matmul(
                                psos[h][:],
                                lhsT=v_all[:, so, pr, 65 * h : 65 * h + 65],
                                rhs=esq[so][:, h, :],
                                start=(so == 0),
                                stop=(so == SC - 1),
                            )
                    pump()
                # softmax denominators: copy l rows to SBUF, move to partition
                # 0, invert, broadcast
                l_sb = p_norm.tile([65, 2, TW], f32, tag="l", name="l_sb")
                for h in range(2):
                    nc.vector.tensor_copy(out=l_sb[64:65, h, :], in_=psos[h][64:65, :])
                lg = p_norm.tile([1, 2, TW], f32, tag="lg", name="lg")
                nc.sync.dma_start(out=lg[0:1, :, :], in_=l_sb[64:65, :, :])
                lginv = p_norm.tile([1, 2, TW], f32, tag="lginv", name="lginv")
                nc.vector.reciprocal_approx_fast(out=lginv[:], in_=lg[:])
                linv = p_norm.tile([64, 2, TW], f32, tag="linv", name="linv")
                for h in range(2):
                    nc.gpsimd.partition_broadcast(
                        out_ap=linv[:, h, :],
                        in_ap=lginv[0:1, h, :],
                        channels=64,
                    )
                for h in range(2):
                    if h == 0:
                        nc.vector.tensor_mul(
                            out=o_allT[0:64, pr, th * TW : (th + 1) * TW],
                            in0=psos[h][0:64, :],
                            in1=linv[:, h, :],
                        )
                    else:
                        ot = p_norm.tile([64, TW], bf16, tag="ot", name="ot")
                        nc.vector.tensor_mul(out=ot[:], in0=psos[h][0:64, :], in1=linv[:, h, :])
                        nc.sync.dma_start(
                            out=o_allT[64:128, pr, th * TW : (th + 1) * TW], in_=ot[:]
                        )

        # ================= emission =================
        # HAM warmup: short N=128 matmuls during the initial DMA wait.
        warm = p_norm.tile([P, P], bf16, tag="warm", name="warm")
        nc.vector.memset(warm[:], 0.0)
        wps = ps_m.tile([P, TW], f32, tag="m", name="wps")
        for i in range(32):
            nc.tensor.matmul(
                wps[:, 0:P], lhsT=warm[:], rhs=warm[:], start=(i == 0), stop=(i == 31)
            )

        # prologue: xt(b0), v(b0) dense, qk(b0, p0) dense
        load_xt(0)
        prep_vall(0)
        for tci in range(TC):
            for g in range(2):
                v_group_unit(0, tci, g)()
        for w in range(2):
            for th in range(NTH):
                qk_unit(0, 0, w, th)()

        for b in range(nb):
            if b + 1 < nb:
                load_xt(b + 1)
                prep_vall(b + 1)
            # y units of the previous batch, spread over this batch's pairs
            ydrip = [[] for _ in range(npair)]
            if b >= 1:
                units = [(tci, j) for tci in range(TC) for j in range(2)]
                for idx, u in enumerate(units):
                    ydrip[min(idx // 4, 4)].append(u)
            for pr in range(npair):
                if pr + 1 < npair:
                    for w in range(2):
                        for th in range(NTH):
                            drip.append(qk_unit(b, pr + 1, w, th))
                elif b + 1 < nb:
                    for w in range(2):
                        for th in range(NTH):
                            drip.append(qk_unit(b + 1, 0, w, th))
                if b + 1 < nb and 1 <= pr <= 4:
                    for tci in (2 * (pr - 1), 2 * (pr - 1) + 1):
                        for g in range(2):
                            drip.append(v_group_unit(b + 1, tci, g))
                for (tci, j) in ydrip[pr]:
                    drip.append(y_unit(b - 1, tci, j))
                attention_pair(b, pr)
        # drain any leftover drip units, then the last batch's y on ps_s banks
        while drip:
            drip.popleft()()
        for tci in range(TC):
            y_unit_tail(nb - 1, tci)()

    nc.compile()
    return nc


class TileOrExit:
    """Combined TileContext + ExitStack context manager."""

    def __init__(self, nc):
        self.nc = nc
        self.ctx = ExitStack()
        self.tc = tile.TileContext(nc)

    def __enter__(self):
        self.ctx.__enter__()
        self.tc.__enter__()
        return self.tc, self.ctx

    def __exit__(self, *a):
        self.ctx.__exit__(*a)
        return self.tc.__exit__(*a)


def prep_inputs(x, Wq, bq, Wk, bk, Wv, bv, Wp, bp, nb, npair, fp8qk=True):
    """Host-side packing into the DRAM layouts the device kernel expects."""
    P = 128
    t = x.shape[1]
    d = x.shape[2]
    KC = d // P
    KC2 = KC // 2
    dpair = 2 * HS

    def to_bf(a):
        return np.ascontiguousarray(a).astype(BF16)

    xt = np.ascontiguousarray(x.transpose(0, 2, 1)).astype(BF16)  # [B, d, t]

    def pack_qk(W):
        # W: [H, d, HS] -> [P, npair, KC, 2*HS]
        w = W.reshape(npair, 2, KC, P, HS)
        w = w.transpose(3, 0, 2, 1, 4).reshape(P, npair, KC, dpair)
        return np.ascontiguousarray(w)

    if fp8qk:
        wq = (pack_qk(Wq) * W8SCALE).reshape(P, npair, KC2, 2, dpair)
        wk = (pack_qk(Wk) * W8SCALE).reshape(P, npair, KC2, 2, dpair)
        wq = np.clip(wq, -240, 240).astype(F8E4)
        wk = np.clip(wk, -240, 240).astype(F8E4)
        xt8 = np.clip(xt.astype(np.float32), -240, 240).astype(F8E4)
        # bias folded as W8SCALE^2 * b at the exp scale; qT/kT hold 32x values,
        # so the additive bias must be 32x too
        bqk = np.stack(
            [bq.reshape(npair, dpair), bk.reshape(npair, dpair)], axis=-1
        ) * W8SCALE
    else:
        wq = to_bf(pack_qk(Wq))
        wk = to_bf(pack_qk(Wk))
        xt8 = None
        bqk = np.stack(
            [bq.reshape(npair, dpair), bk.reshape(npair, dpair)], axis=-1
        )
    wv = pack_qk(Wv).transpose(0, 2, 1, 3).reshape(P, KC, npair * dpair)
    wv = to_bf(wv)
    wp = to_bf(Wp.reshape(KC, P, d).transpose(1, 0, 2))
    bqk = np.ascontiguousarray(bqk.transpose(1, 0, 2)).astype(np.float32)
    bv_bc = to_bf(np.broadcast_to(bv.reshape(1, npair, dpair), (P, npair, dpair)))
    bp_bc = np.ascontiguousarray(np.broadcast_to(bp.reshape(1, d), (P, d))).astype(
        np.float32
    )

    weights = {
        "wq": wq, "wk": wk, "wv": wv, "wp": wp,
        "bqk": bqk, "bv": bv_bc, "bp": bp_bc,
    }
    n_cores = x.shape[0] // nb
    in_maps = []
    for i in range(n_cores):
        m = dict(weights)
        m["xt"] = np.ascontiguousarray(xt[i * nb : (i + 1) * nb])
        if fp8qk:
            m["xt8"] = np.ascontiguousarray(xt8[i * nb : (i + 1) * nb])
        in_maps.append(m)
    return in_maps


_NC_CACHE = {}
LAST_RESULT = {}
VARIANT = os.environ.get("MHA_VARIANT", "fp8qk")


def kernel(x, Wq, bq, Wk, bk, Wv, bv, Wp, bp, _trace=False):
    x = np.asarray(x, dtype=np.float32)
    Wq, bq = np.asarray(Wq, np.float32), np.asarray(bq, np.float32)
    Wk, bk = np.asarray(Wk, np.float32), np.asarray(bk, np.float32)
    Wv, bv = np.asarray(Wv, np.float32), np.asarray(bv, np.float32)
    Wp, bp = np.asarray(Wp, np.float32), np.asarray(bp, np.float32)

    npair = H // 2
    key = (VARIANT, NB, T_FULL, D_FULL, npair)
    if key not in _NC_CACHE:
        _NC_CACHE[key] = build_mha_nc(NB, T_FULL, D_FULL, npair, variant=VARIANT)
    nc = _NC_CACHE[key]

    in_maps = prep_inputs(
        x, Wq, bq, Wk, bk, Wv, bv, Wp, bp, NB, npair, fp8qk="fp8qk" in VARIANT
    )
    res = run_bass_kernel_spmd(
        nc, in_maps, core_ids=list(range(N_CORES)), trace=_trace
    )
    LAST_RESULT["exec_time_ns"] = res.exec_time_ns
    LAST_RESULT["res"] = res
    outs = [res.results[i]["y"] for i in range(N_CORES)]
    return np.concatenate(outs, axis=0).astype(np.float32)


# revision 7
# speedup vs baseline: 1.0930x; 1.0930x over previous
"""Multi-head attention (B=16, T=1024, D=768, H=12) on 8 TRN2 NeuronCores.

Strategy: pure data parallelism over the batch — each core computes full MHA
for 2 batch elements. No collectives.

v2 changes over the 381us baseline:
  - q/k projections run as fp8e4 DoubleRow matmuls (weights pre-scaled x32 on
    host, bias folded as 32*b, exp scale divided by 32*32). v/O/y stay bf16.
  - All projection work (v-proj, q/k-proj, y-proj) is emitted through a drip
    queue that releases ~1 unit per attention sc-iteration, so the exp-bound
    middle absorbs projection matmuls in its PE slack instead of running them
    as dense PE-only phases.
  - Lean HAM warmup (N=128 matmuls instead of N=512).
  - ps_o gets 3 PSUM banks (th-boundary overlap), drip pool drops to 1.
  - l (softmax denominator) rows leave PSUM by direct DMA (no DVE copy).
  - y(last batch) runs on the ps_s pool banks after attention finishes.
"""

import os
from collections import deque
from contextlib import ExitStack

import numpy as np
import ml_dtypes

import concourse.bacc as bacc
import concourse.bass as bass
import concourse.mybir as mybir
import concourse.tile as tile
from concourse.bass_utils import run_bass_kernel_spmd

BF16 = ml_dtypes.bfloat16
F8E4 = ml_dtypes.float8_e4m3

# Full problem dims
B, T_FULL, D_FULL, H, HS = 16, 1024, 768, 12, 64
N_CORES = 8
NB = B // N_CORES  # batch elements per core
W8SCALE = 32.0     # host pre-scale on Wq/Wk before fp8 cast


def build_mha_nc(nb, t, d, npair, trn_type="TRN2", variant="fp8qk"):
    P = 128
    KC = d // P              # contraction chunks over model dim
    KC2 = KC // 2            # DoubleRow chunk pairs
    SC = t // P              # s (key position) chunks
    NTH = max(1, t // 512)   # output-column groups for S/O matmuls
    TW = t // NTH            # width of each group (<= 512)
    TC = t // P              # t row chunks for v/y
    D2 = d // 2              # y-proj free-dim split (<= 512 fp32 psum)
    dpair = 2 * HS           # 128
    fp8qk = "fp8qk" in variant
    scale = 1.0 / np.sqrt(HS)
    if fp8qk:
        scale /= W8SCALE * W8SCALE

    f32 = mybir.dt.float32
    bf16 = mybir.dt.bfloat16
    f8e4 = mybir.dt.float8e4
    AF = mybir.ActivationFunctionType
    DR = mybir.MatmulPerfMode.DoubleRow

    nc = bacc.Bacc(trn_type, target_bir_lowering=False, debug=False)

    xt_d = nc.dram_tensor("xt", [nb, d, t], bf16, kind="ExternalInput").ap()
    if fp8qk:
        xt8_d = nc.dram_tensor("xt8", [nb, d, t], f8e4, kind="ExternalInput").ap()
        wq_d = nc.dram_tensor("wq", [P, npair, KC2, 2, dpair], f8e4, kind="ExternalInput").ap()
        wk_d = nc.dram_tensor("wk", [P, npair, KC2, 2, dpair], f8e4, kind="ExternalInput").ap()
    else:
        wq_d = nc.dram_tensor("wq", [P, npair, KC, dpair], bf16, kind="ExternalInput").ap()
        wk_d = nc.dram_tensor("wk", [P, npair, KC, dpair], bf16, kind="ExternalInput").ap()
    wv_d = nc.dram_tensor("wv", [P, KC, npair * dpair], bf16, kind="ExternalInput").ap()
    wp_d = nc.dram_tensor("wp", [P, KC, d], bf16, kind="ExternalInput").ap()
    bqk_d = nc.dram_tensor("bqk", [P, npair, 2], f32, kind="ExternalInput").ap()
    bv_d = nc.dram_tensor("bv", [P, npair, dpair], bf16, kind="ExternalInput").ap()
    bp_d = nc.dram_tensor("bp", [P, d], f32, kind="ExternalInput").ap()
    y_d = nc.dram_tensor("y", [nb, t, d], f32, kind="ExternalOutput").ap()

    with TileOrExit(nc) as (tc, ctx):
        # ---- persistent weights ----
        p_w = ctx.enter_context(tc.tile_pool(name="p_w", bufs=1))
        if fp8qk:
            wq_sb = p_w.tile([P, npair, KC2, 2, dpair], f8e4, tag="wq", name="wq_sb")
            wk_sb = p_w.tile([P, npair, KC2, 2, dpair], f8e4, tag="wk", name="wk_sb")
        else:
            wq_sb = p_w.tile([P, npair, KC, dpair], bf16, tag="wq", name="wq_sb")
            wk_sb = p_w.tile([P, npair, KC, dpair], bf16, tag="wk", name="wk_sb")
        wv_sb = p_w.tile([P, KC, npair * dpair], bf16, tag="wv", name="wv_sb")
        wp_sb = p_w.tile([P, KC, d], bf16, tag="wp", name="wp_sb")
        bqk_sb = p_w.tile([P, npair, 2], f32, tag="bqk", name="bqk_sb")
        bv_sb = p_w.tile([P, npair, dpair], bf16, tag="bv", name="bv_sb")
        bp_sb = p_w.tile([P, d], f32, tag="bp", name="bp_sb")
        for c in range(KC):
            nc.gpsimd.dma_start(wv_sb[:, c], wv_d[:, c])
        nc.gpsimd.dma_start(bv_sb[:], bv_d)
        for pr in range(npair):
            nc.gpsimd.dma_start(wq_sb[:, pr], wq_d[:, pr])
            nc.gpsimd.dma_start(wk_sb[:, pr], wk_d[:, pr])
        nc.gpsimd.dma_start(bqk_sb[:], bqk_d)
        nc.gpsimd.dma_start(wp_sb[:], wp_d)
        nc.gpsimd.dma_start(bp_sb[:], bp_d)

        # ---- pools ----
        p_xt = ctx.enter_context(tc.tile_pool(name="p_xt", bufs=2))
        if fp8qk:
            p_xt8 = ctx.enter_context(tc.tile_pool(name="p_xt8", bufs=2))
        p_vall = ctx.enter_context(tc.tile_pool(name="p_vall", bufs=2))
        p_qk = ctx.enter_context(tc.tile_pool(name="p_qk", bufs=3))
        p_es = ctx.enter_context(tc.tile_pool(name="p_es", bufs=4))
        p_oall = ctx.enter_context(tc.tile_pool(name="p_oall", bufs=1))
        p_norm = ctx.enter_context(tc.tile_pool(name="p_norm", bufs=2))
        p_y = ctx.enter_context(tc.tile_pool(name="p_y", bufs=2))
        # PSUM: ps_s 2x2 banks, ps_o 2x1, ps_m 2x1 => 8 banks
        ps_s = ctx.enter_context(tc.tile_pool(name="ps_s", bufs=2, space="PSUM"))
        ps_o = ctx.enter_context(tc.tile_pool(name="ps_o", bufs=2, space="PSUM"))
        ps_m = ctx.enter_context(tc.tile_pool(name="ps_m", bufs=2, space="PSUM"))

        xts = [None] * nb
        xt8s = [None] * nb
        v_alls = [None] * nb
        o_allTs = [None] * nb
        qkTs = {}   # (b, pr) -> (qT, kT)
        y_sbs = {}  # (b, tci) -> y_sb

        def load_xt(b):
            xt = p_xt.tile([P, KC, t], bf16, tag="xt", name="xt_sb")
            xt_src = xt_d[b].rearrange("(c p) t -> p c t", p=P)
            for c in range(KC):
                nc.sync.dma_start(xt[:, c], xt_src[:, c])
            xts[b] = xt
            if fp8qk:
                xt8 = p_xt8.tile([P, KC, t], f8e4, tag="xt8", name="xt8_sb")
                xt8_src = xt8_d[b].rearrange("(c p) t -> p c t", p=P)
                for c in range(KC):
                    nc.sync.dma_start(xt8[:, c], xt8_src[:, c])
                xt8s[b] = xt8

        def prep_vall(b):
            v_alls[b] = p_vall.tile([P, SC, npair, 130], bf16, tag="vall", name="v_all")
            ones_view = v_alls[b].rearrange("p s r (h x) -> p s r h x", h=2)
            nc.gpsimd.memset(ones_view[:, :, :, :, 64:65], 1.0)
            o_allTs[b] = p_oall.tile([P, npair, t], bf16, tag=f"oall{b}", name="o_allT")

        # ---------- drip units ----------
        def v_group_unit(b, tci, g, gn=3):
            def run():
                xt = xts[b]
                psv = ps_m.tile([P, TW], f32, tag="m", name="psv")
                for c in range(KC):
                    nc.tensor.matmul(
                        psv[:, : gn * dpair],
                        lhsT=xt[:, c, tci * P : (tci + 1) * P],
                        rhs=wv_sb[:, c, 3 * g * dpair : (3 * g + gn) * dpair],
                        start=(c == 0),
                        stop=(c == KC - 1),
                    )
                glo = 3 * g
                dst = v_alls[b][:, tci, glo : glo + gn, :].rearrange(
                    "p r (h x) -> p r h x", h=2
                )[:, :, :, 0:64]
                src = psv[:, : gn * dpair].rearrange("p (r h e) -> p r h e", r=gn, h=2)
                bias = bv_sb[:, glo : glo + gn, :].rearrange("p r (h e) -> p r h e", h=2)
                nc.vector.tensor_add(out=dst, in0=src, in1=bias)
            return run

        def qk_unit(b, pr, which, th):
            def run():
                if (b, pr) not in qkTs:
                    qkTs[(b, pr)] = (
                        p_qk.tile([P, t], bf16, tag="qT", name="qT"),
                        p_qk.tile([P, t], bf16, tag="kT", name="kT"),
                    )
                w_sb = wq_sb if which == 0 else wk_sb
                dstT = qkTs[(b, pr)][which]
                psq = ps_m.tile([P, TW], f32, tag="m", name="psq")
                if fp8qk:
                    xt8 = xt8s[b]
                    for i in range(KC2):
                        nc.tensor.matmul(
                            psq[:],
                            lhsT=w_sb[:, pr, i, :, :],
                            rhs=xt8[:, 2 * i : 2 * i + 2, th * TW : (th + 1) * TW],
                            start=(i == 0),
                            stop=(i == KC2 - 1),
                            perf_mode=DR,
                        )
                else:
                    xt = xts[b]
                    for c in range(KC):
                        nc.tensor.matmul(
                            psq[:],
                            lhsT=w_sb[:, pr, c, :],
                            rhs=xt[:, c, th * TW : (th + 1) * TW],
                            start=(c == 0),
                            stop=(c == KC - 1),
                        )
                nc.vector.tensor_scalar_add(
                    out=dstT[:, th * TW : (th + 1) * TW],
                    in0=psq[:],
                    scalar1=bqk_sb[:, pr, which : which + 1],
                )
            return run

        def y_unit(b, tci, j):
            def run():
                if (b, tci) not in y_sbs:
                    y_sbs[(b, tci)] = p_y.tile([P, d], f32, tag="y", name="y_sb")
                y_sb = y_sbs[(b, tci)]
                psy = ps_m.tile([P, TW], f32, tag="m", name="psy")
                for c in range(KC):
                    nc.tensor.matmul(
                        psy[:, 0:D2],
                        lhsT=o_allTs[b][:, c, tci * P : (tci + 1) * P],
                        rhs=wp_sb[:, c, j * D2 : (j + 1) * D2],
                        start=(c == 0),
                        stop=(c == KC - 1),
                    )
                nc.vector.tensor_add(
                    out=y_sb[:, j * D2 : (j + 1) * D2],
                    in0=psy[:, 0:D2],
                    in1=bp_sb[:, j * D2 : (j + 1) * D2],
                )
                if j == 1:
                    nc.sync.dma_start(out=y_d[b, tci * P : (tci + 1) * P, :], in_=y_sb[:])
            return run

        def y_unit_tail(b, tci):
            def run():
                psy = ps_s.tile([P, 2, TW], f32, tag="s", name="ps_s")
                for c in range(KC):
                    for j in range(2):
                        nc.tensor.matmul(
                            psy[:, j, 0:D2],
                            lhsT=o_allTs[b][:, c, tci * P : (tci + 1) * P],
                            rhs=wp_sb[:, c, j * D2 : (j + 1) * D2],
                            start=(c == 0),
                            stop=(c == KC - 1),
                        )
                y_sb = p_y.tile([P, d], f32, tag="y", name="y_sb")
                for j in range(2):
                    nc.vector.tensor_add(
                        out=y_sb[:, j * D2 : (j + 1) * D2],
                        in0=psy[:, j, 0:D2],
                        in1=bp_sb[:, j * D2 : (j + 1) * D2],
                    )
                nc.sync.dma_start(out=y_d[b, tci * P : (tci + 1) * P, :], in_=y_sb[:])
            return run

        drip = deque()

        def pump():
            if drip:
                drip.popleft()()

        # ---------- attention middle for one (b, pair) ----------
        def attention_pair(b, pr):
            qT, kT = qkTs[(b, pr)]
            v_all = v_alls[b]
            o_allT = o_allTs[b]
            for th in range(NTH):
                esq = [None] * SC
                psos = [ps_o.tile([65, TW], f32, tag="o", name="pso") for _ in range(2)]
                for sc in range(SC + 2):
                    if sc < SC:
                        esq[sc] = p_es.tile([P, 2, TW], bf16, tag="es", name="es")
                        ps = ps_s.tile([P, 2, TW], f32, tag="s", name="ps_s")
                        nc.tensor.matmul(
                            ps[:, 0, :],
                            lhsT=kT[0:64, sc * P : (sc + 1) * P],
                            rhs=qT[0:64, th * TW : (th + 1) * TW],
                            start=True,
                            stop=True,
                        )
                        nc.tensor.matmul(
                            ps[:, 1, :],
                            lhsT=kT[64:128, sc * P : (sc + 1) * P],
                            rhs=qT[64:128, th * TW : (th + 1) * TW],
                            start=True,
                            stop=True,
                            tile_position=(64, 0),
                        )
                        nc.scalar.activation(
                            out=esq[sc][:], in_=ps[:], func=AF.Exp, scale=scale
                        )
                    if sc >= 2:
                        so = sc - 2
                        for h in range(2):
                            nc.tensor.matmul(
                                psos[h][:],
                                lhsT=v_all[:, so, pr, 65 * h : 65 * h + 65],
                                rhs=esq[so][:, h, :],
                                start=(so == 0),
                                stop=(so == SC - 1),
                            )
                    pump()
                # softmax denominators: copy l rows to SBUF, move to partition
                # 0, invert, broadcast
                l_sb = p_norm.tile([65, 2, TW], f32, tag="l", name="l_sb")
                for h in range(2):
                    nc.vector.tensor_copy(out=l_sb[64:65, h, :], in_=psos[h][64:65, :])
                lg = p_norm.tile([1, 2, TW], f32, tag="lg", name="lg")
                nc.sync.dma_start(out=lg[0:1, :, :], in_=l_sb[64:65, :, :])
                lginv = p_norm.tile([1, 2, TW], f32, tag="lginv", name="lginv")
                nc.vector.reciprocal_approx_fast(out=lginv[:], in_=lg[:])
                linv = p_norm.tile([64, 2, TW], f32, tag="linv", name="linv")
                for h in range(2):
                    nc.gpsimd.partition_broadcast(
                        out_ap=linv[:, h, :],
                        in_ap=lginv[0:1, h, :],
                        channels=64,
                    )
                for h in range(2):
                    if h == 0:
                        nc.vector.tensor_mul(
                            out=o_allT[0:64, pr, th * TW : (th + 1) * TW],
                            in0=psos[h][0:64, :],
                            in1=linv[:, h, :],
                        )
                    else:
                        ot = p_norm.tile([64, TW], bf16, tag="ot", name="ot")
                        nc.vector.tensor_mul(out=ot[:], in0=psos[h][0:64, :], in1=linv[:, h, :])
                        nc.sync.dma_start(
                            out=o_allT[64:128, pr, th * TW : (th + 1) * TW], in_=ot[:]
                        )

        # ================= emission =================
        # HAM warmup: short N=128 matmuls during the initial DMA wait.
        warm = p_norm.tile([P, P], bf16, tag="warm", name="warm")
        nc.vector.memset(warm[:], 0.0)
        wps = ps_m.tile([P, TW], f32, tag="m", name="wps")
        for i in range(32):
            nc.tensor.matmul(
                wps[:, 0:P], lhsT=warm[:], rhs=warm[:], start=(i == 0), stop=(i == 31)
            )

        # prologue: xt(b0), v(b0) dense, qk(b0, p0) dense
        load_xt(0)
        prep_vall(0)
        for tci in range(TC):
            for g in range(2):
                v_group_unit(0, tci, g)()
        for w in range(2):
            for th in range(NTH):
                qk_unit(0, 0, w, th)()

        for b in range(nb):
            if b + 1 < nb:
                load_xt(b + 1)
                prep_vall(b + 1)
            # y units of the previous batch, spread over this batch's pairs
            ydrip = [[] for _ in range(npair)]
            if b >= 1:
                units = [(tci, j) for tci in range(TC) for j in range(2)]
                alloc = [2, 3, 3, 3, 3, 2]
                k = 0
                for p in range(npair):
                    for _ in range(alloc[p]):
                        ydrip[p].append(units[k]); k += 1
            for pr in range(npair):
                if pr + 1 < npair:
                    for w in range(2):
                        for th in range(NTH):
                            drip.append(qk_unit(b, pr + 1, w, th))
                elif b + 1 < nb:
                    for w in range(2):
                        for th in range(NTH):
                            drip.append(qk_unit(b + 1, 0, w, th))
                if b + 1 < nb and 1 <= pr <= 4:
                    for tci in (2 * (pr - 1), 2 * (pr - 1) + 1):
                        for g in range(2):
                            drip.append(v_group_unit(b + 1, tci, g))
                for (tci, j) in ydrip[pr]:
                    drip.append(y_unit(b - 1, tci, j))
                attention_pair(b, pr)
        # drain any leftover drip units, then the last batch's y on ps_s banks
        while drip:
            drip.popleft()()
        for tci in range(TC):
            y_unit_tail(nb - 1, tci)()

    nc.compile()
    return nc


class TileOrExit:
    """Combined TileContext + ExitStack context manager."""

    def __init__(self, nc):
        self.nc = nc
        self.ctx = ExitStack()
        self.tc = tile.TileContext(nc)

    def __enter__(self):
        self.ctx.__enter__()
        self.tc.__enter__()
        return self.tc, self.ctx

    def __exit__(self, *a):
        self.ctx.__exit__(*a)
        return self.tc.__exit__(*a)


def prep_inputs(x, Wq, bq, Wk, bk, Wv, bv, Wp, bp, nb, npair, fp8qk=True):
    """Host-side packing into the DRAM layouts the device kernel expects."""
    P = 128
    t = x.shape[1]
    d = x.shape[2]
    KC = d // P
    KC2 = KC // 2
    dpair = 2 * HS

    def to_bf(a):
        return np.ascontiguousarray(a).astype(BF16)

    xt = np.ascontiguousarray(x.transpose(0, 2, 1)).astype(BF16)  # [B, d, t]

    def pack_qk(W):
        # W: [H, d, HS] -> [P, npair, KC, 2*HS]
        w = W.reshape(npair, 2, KC, P, HS)
        w = w.transpose(3, 0, 2, 1, 4).reshape(P, npair, KC, dpair)
        return np.ascontiguousarray(w)

    if fp8qk:
        wq = (pack_qk(Wq) * W8SCALE).reshape(P, npair, KC2, 2, dpair)
        wk = (pack_qk(Wk) * W8SCALE).reshape(P, npair, KC2, 2, dpair)
        wq = np.clip(wq, -240, 240).astype(F8E4)
        wk = np.clip(wk, -240, 240).astype(F8E4)
        xt8 = np.clip(xt.astype(np.float32), -240, 240).astype(F8E4)
        # bias folded as W8SCALE^2 * b at the exp scale; qT/kT hold 32x values,
        # so the additive bias must be 32x too
        bqk = np.stack(
            [bq.reshape(npair, dpair), bk.reshape(npair, dpair)], axis=-1
        ) * W8SCALE
    else:
        wq = to_bf(pack_qk(Wq))
        wk = to_bf(pack_qk(Wk))
        xt8 = None
        bqk = np.stack(
            [bq.reshape(npair, dpair), bk.reshape(npair, dpair)], axis=-1
        )
    wv = pack_qk(Wv).transpose(0, 2, 1, 3).reshape(P, KC, npair * dpair)
    wv = to_bf(wv)
    wp = to_bf(Wp.reshape(KC, P, d).transpose(1, 0, 2))
    bqk = np.ascontiguousarray(bqk.transpose(1, 0, 2)).astype(np.float32)
    bv_bc = to_bf(np.broadcast_to(bv.reshape(1, npair, dpair), (P, npair, dpair)))
    bp_bc = np.ascontiguousarray(np.broadcast_to(bp.reshape(1, d), (P, d))).astype(
        np.float32
    )

    weights = {
        "wq": wq, "wk": wk, "wv": wv, "wp": wp,
        "bqk": bqk, "bv": bv_bc, "bp": bp_bc,
    }
    n_cores = x.shape[0] // nb
    in_maps = []
    for i in range(n_cores):
        m = dict(weights)
        m["xt"] = np.ascontiguousarray(xt[i * nb : (i + 1) * nb])
        if fp8qk:
            m["xt8"] = np.ascontiguousarray(xt8[i * nb : (i + 1) * nb])
        in_maps.append(m)
    return in_maps


_NC_CACHE = {}
LAST_RESULT = {}
VARIANT = os.environ.get("MHA_VARIANT", "fp8qk")


def kernel(x, Wq, bq, Wk, bk, Wv, bv, Wp, bp, _trace=False):
    x = np.asarray(x, dtype=np.float32)
    Wq, bq = np.asarray(Wq, np.float32), np.asarray(bq, np.float32)
    Wk, bk = np.asarray(Wk, np.float32), np.asarray(bk, np.float32)
    Wv, bv = np.asarray(Wv, np.float32), np.asarray(bv, np.float32)
    Wp, bp = np.asarray(Wp, np.float32), np.asarray(bp, np.float32)

    npair = H // 2
    key = (VARIANT, NB, T_FULL, D_FULL, npair)
    if key not in _NC_CACHE:
        _NC_CACHE[key] = build_mha_nc(NB, T_FULL, D_FULL, npair, variant=VARIANT)
    nc = _NC_CACHE[key]

    in_maps = prep_inputs(
        x, Wq, bq, Wk, bk, Wv, bv, Wp, bp, NB, npair, fp8qk="fp8qk" in VARIANT
    )
    res = run_bass_kernel_spmd(
        nc, in_maps, core_ids=list(range(N_CORES)), trace=_trace
    )
    LAST_RESULT["exec_time_ns"] = res.exec_time_ns
    LAST_RESULT["res"] = res
    outs = [res.results[i]["y"] for i in range(N_CORES)]
    return np.concatenate(outs, axis=0).astype(np.float32)


# revision 8
# speedup vs baseline: 1.2185x; 1.1148x over previous
"""Multi-head attention (B=16, T=1024, D=768, H=12) on 8 TRN2 NeuronCores.

Strategy: pure data parallelism over the batch — each core computes full MHA
for 2 batch elements. No collectives.

v2 changes over the 381us baseline:
  - q/k projections run as fp8e4 DoubleRow matmuls (weights pre-scaled x32 on
    host, bias folded as 32*b, exp scale divided by 32*32). v/O/y stay bf16.
  - All projection work (v-proj, q/k-proj, y-proj) is emitted through a drip
    queue that releases ~1 unit per attention sc-iteration, so the exp-bound
    middle absorbs projection matmuls in its PE slack instead of running them
    as dense PE-only phases.
  - Lean HAM warmup (N=128 matmuls instead of N=512).
  - ps_o gets 3 PSUM banks (th-boundary overlap), drip pool drops to 1.
  - l (softmax denominator) rows leave PSUM by direct DMA (no DVE copy).
  - y(last batch) runs on the ps_s pool banks after attention finishes.
"""

import os
from collections import deque
from contextlib import ExitStack

import numpy as np
import ml_dtypes

import concourse.bacc as bacc
import concourse.bass as bass
import concourse.mybir as mybir
import concourse.tile as tile
from concourse.bass_utils import run_bass_kernel_spmd

BF16 = ml_dtypes.bfloat16
F8E4 = ml_dtypes.float8_e4m3

# Full problem dims
B, T_FULL, D_FULL, H, HS = 16, 1024, 768, 12, 64
N_CORES = 8
NB = B // N_CORES  # batch elements per core
W8SCALE = 32.0     # host pre-scale on Wq/Wk before fp8 cast


def build_mha_nc(nb, t, d, npair, trn_type="TRN2", variant="fp8qk"):
    P = 128
    KC = d // P              # contraction chunks over model dim
    KC2 = KC // 2            # DoubleRow chunk pairs
    SC = t // P              # s (key position) chunks
    NTH = max(1, t // 512)   # output-column groups for S/O matmuls
    TW = t // NTH            # width of each group (<= 512)
    TC = t // P              # t row chunks for v/y
    D2 = d // 2              # y-proj free-dim split (<= 512 fp32 psum)
    dpair = 2 * HS           # 128
    fp8qk = "fp8qk" in variant
    scale = 1.0 / np.sqrt(HS)
    if fp8qk:
        scale /= W8SCALE * W8SCALE

    f32 = mybir.dt.float32
    bf16 = mybir.dt.bfloat16
    f8e4 = mybir.dt.float8e4
    AF = mybir.ActivationFunctionType
    DR = mybir.MatmulPerfMode.DoubleRow

    nc = bacc.Bacc(trn_type, target_bir_lowering=False, debug=False)

    xt_d = nc.dram_tensor("xt", [nb, d, t], bf16, kind="ExternalInput").ap()
    if fp8qk:
        xt8_d = nc.dram_tensor("xt8", [nb, d, t], f8e4, kind="ExternalInput").ap()
        wq_d = nc.dram_tensor("wq", [P, npair, KC2, 2, dpair], f8e4, kind="ExternalInput").ap()
        wk_d = nc.dram_tensor("wk", [P, npair, KC2, 2, dpair], f8e4, kind="ExternalInput").ap()
    else:
        wq_d = nc.dram_tensor("wq", [P, npair, KC, dpair], bf16, kind="ExternalInput").ap()
        wk_d = nc.dram_tensor("wk", [P, npair, KC, dpair], bf16, kind="ExternalInput").ap()
    wv_d = nc.dram_tensor("wv", [P, KC, npair * dpair], bf16, kind="ExternalInput").ap()
    wp_d = nc.dram_tensor("wp", [P, KC, d], bf16, kind="ExternalInput").ap()
    bqk_d = nc.dram_tensor("bqk", [P, npair, 2], f32, kind="ExternalInput").ap()
    bv_d = nc.dram_tensor("bv", [P, npair, dpair], bf16, kind="ExternalInput").ap()
    bp_d = nc.dram_tensor("bp", [P, d], f32, kind="ExternalInput").ap()
    y_d = nc.dram_tensor("y", [nb, t, d], f32, kind="ExternalOutput").ap()

    with TileOrExit(nc) as (tc, ctx):
        # ---- persistent weights ----
        p_w = ctx.enter_context(tc.tile_pool(name="p_w", bufs=1))
        if fp8qk:
            wq_sb = p_w.tile([P, npair, KC2, 2, dpair], f8e4, tag="wq", name="wq_sb")
            wk_sb = p_w.tile([P, npair, KC2, 2, dpair], f8e4, tag="wk", name="wk_sb")
        else:
            wq_sb = p_w.tile([P, npair, KC, dpair], bf16, tag="wq", name="wq_sb")
            wk_sb = p_w.tile([P, npair, KC, dpair], bf16, tag="wk", name="wk_sb")
        wv_sb = p_w.tile([P, KC, npair * dpair], bf16, tag="wv", name="wv_sb")
        wp_sb = p_w.tile([P, KC, d], bf16, tag="wp", name="wp_sb")
        bqk_sb = p_w.tile([P, npair, 2], f32, tag="bqk", name="bqk_sb")
        bv_sb = p_w.tile([P, npair, dpair], bf16, tag="bv", name="bv_sb")
        bp_sb = p_w.tile([P, d], f32, tag="bp", name="bp_sb")
        for c in range(KC):
            nc.gpsimd.dma_start(wv_sb[:, c], wv_d[:, c])
        nc.gpsimd.dma_start(bv_sb[:], bv_d)
        for pr in range(npair):
            nc.gpsimd.dma_start(wq_sb[:, pr], wq_d[:, pr])
            nc.gpsimd.dma_start(wk_sb[:, pr], wk_d[:, pr])
        nc.gpsimd.dma_start(bqk_sb[:], bqk_d)
        nc.gpsimd.dma_start(wp_sb[:], wp_d)
        nc.gpsimd.dma_start(bp_sb[:], bp_d)

        # ---- pools ----
        p_xt = ctx.enter_context(tc.tile_pool(name="p_xt", bufs=2))
        if fp8qk:
            p_xt8 = ctx.enter_context(tc.tile_pool(name="p_xt8", bufs=2))
        p_vall = ctx.enter_context(tc.tile_pool(name="p_vall", bufs=2))
        p_qk = ctx.enter_context(tc.tile_pool(name="p_qk", bufs=3))
        p_es = ctx.enter_context(tc.tile_pool(name="p_es", bufs=4))
        p_oall = ctx.enter_context(tc.tile_pool(name="p_oall", bufs=1))
        p_norm = ctx.enter_context(tc.tile_pool(name="p_norm", bufs=2))
        p_y = ctx.enter_context(tc.tile_pool(name="p_y", bufs=2))
        # PSUM: ps_s 2x2 banks, ps_o 3x1, ps_m 1x1 => 8 banks
        ps_s = ctx.enter_context(tc.tile_pool(name="ps_s", bufs=2, space="PSUM"))
        ps_o = ctx.enter_context(tc.tile_pool(name="ps_o", bufs=3, space="PSUM"))
        ps_m = ctx.enter_context(tc.tile_pool(name="ps_m", bufs=1, space="PSUM"))

        xts = [None] * nb
        xt8s = [None] * nb
        v_alls = [None] * nb
        o_allTs = [None] * nb
        qkTs = {}   # (b, pr) -> (qT, kT)
        y_sbs = {}  # (b, tci) -> y_sb

        def load_xt(b):
            xt = p_xt.tile([P, KC, t], bf16, tag="xt", name="xt_sb")
            xt_src = xt_d[b].rearrange("(c p) t -> p c t", p=P)
            for c in range(KC):
                nc.sync.dma_start(xt[:, c], xt_src[:, c])
            xts[b] = xt
            if fp8qk:
                xt8 = p_xt8.tile([P, KC, t], f8e4, tag="xt8", name="xt8_sb")
                xt8_src = xt8_d[b].rearrange("(c p) t -> p c t", p=P)
                for c in range(KC):
                    nc.sync.dma_start(xt8[:, c], xt8_src[:, c])
                xt8s[b] = xt8

        def prep_vall(b):
            v_alls[b] = p_vall.tile([P, SC, npair, 130], bf16, tag="vall", name="v_all")
            ones_view = v_alls[b].rearrange("p s r (h x) -> p s r h x", h=2)
            nc.gpsimd.memset(ones_view[:, :, :, :, 64:65], 1.0)
            o_allTs[b] = p_oall.tile([P, npair, t], bf16, tag=f"oall{b}", name="o_allT")

        # ---------- drip units ----------
        def v_group_unit(b, tci, g, gn=3):
            def run():
                xt = xts[b]
                psv = ps_m.tile([P, TW], f32, tag="m", name="psv")
                for c in range(KC):
                    nc.tensor.matmul(
                        psv[:, : gn * dpair],
                        lhsT=xt[:, c, tci * P : (tci + 1) * P],
                        rhs=wv_sb[:, c, 3 * g * dpair : (3 * g + gn) * dpair],
                        start=(c == 0),
                        stop=(c == KC - 1),
                    )
                glo = 3 * g
                dst = v_alls[b][:, tci, glo : glo + gn, :].rearrange(
                    "p r (h x) -> p r h x", h=2
                )[:, :, :, 0:64]
                src = psv[:, : gn * dpair].rearrange("p (r h e) -> p r h e", r=gn, h=2)
                bias = bv_sb[:, glo : glo + gn, :].rearrange("p r (h e) -> p r h e", h=2)
                nc.vector.tensor_add(out=dst, in0=src, in1=bias)
            return run

        def qk_unit(b, pr, which, th):
            def run():
                if (b, pr) not in qkTs:
                    qkTs[(b, pr)] = (
                        p_qk.tile([P, t], bf16, tag="qT", name="qT"),
                        p_qk.tile([P, t], bf16, tag="kT", name="kT"),
                    )
                w_sb = wq_sb if which == 0 else wk_sb
                dstT = qkTs[(b, pr)][which]
                psq = ps_m.tile([P, TW], f32, tag="m", name="psq")
                if fp8qk:
                    xt8 = xt8s[b]
                    for i in range(KC2):
                        nc.tensor.matmul(
                            psq[:],
                            lhsT=w_sb[:, pr, i, :, :],
                            rhs=xt8[:, 2 * i : 2 * i + 2, th * TW : (th + 1) * TW],
                            start=(i == 0),
                            stop=(i == KC2 - 1),
                            perf_mode=DR,
                        )
                else:
                    xt = xts[b]
                    for c in range(KC):
                        nc.tensor.matmul(
                            psq[:],
                            lhsT=w_sb[:, pr, c, :],
                            rhs=xt[:, c, th * TW : (th + 1) * TW],
                            start=(c == 0),
                            stop=(c == KC - 1),
                        )
                nc.vector.tensor_scalar_add(
                    out=dstT[:, th * TW : (th + 1) * TW],
                    in0=psq[:],
                    scalar1=bqk_sb[:, pr, which : which + 1],
                )
            return run

        def y_unit(b, tci, j):
            def run():
                if (b, tci) not in y_sbs:
                    y_sbs[(b, tci)] = p_y.tile([P, d], f32, tag="y", name="y_sb")
                y_sb = y_sbs[(b, tci)]
                psy = ps_m.tile([P, TW], f32, tag="m", name="psy")
                for c in range(KC):
                    nc.tensor.matmul(
                        psy[:, 0:D2],
                        lhsT=o_allTs[b][:, c, tci * P : (tci + 1) * P],
                        rhs=wp_sb[:, c, j * D2 : (j + 1) * D2],
                        start=(c == 0),
                        stop=(c == KC - 1),
                    )
                nc.vector.tensor_add(
                    out=y_sb[:, j * D2 : (j + 1) * D2],
                    in0=psy[:, 0:D2],
                    in1=bp_sb[:, j * D2 : (j + 1) * D2],
                )
                if j == 1:
                    nc.sync.dma_start(out=y_d[b, tci * P : (tci + 1) * P, :], in_=y_sb[:])
            return run

        def y_unit_tail(b, tci):
            def run():
                psy = ps_s.tile([P, 2, TW], f32, tag="s", name="ps_s")
                for c in range(KC):
                    for j in range(2):
                        nc.tensor.matmul(
                            psy[:, j, 0:D2],
                            lhsT=o_allTs[b][:, c, tci * P : (tci + 1) * P],
                            rhs=wp_sb[:, c, j * D2 : (j + 1) * D2],
                            start=(c == 0),
                            stop=(c == KC - 1),
                        )
                y_sb = p_y.tile([P, d], f32, tag="y", name="y_sb")
                for j in range(2):
                    nc.vector.tensor_add(
                        out=y_sb[:, j * D2 : (j + 1) * D2],
                        in0=psy[:, j, 0:D2],
                        in1=bp_sb[:, j * D2 : (j + 1) * D2],
                    )
                nc.sync.dma_start(out=y_d[b, tci * P : (tci + 1) * P, :], in_=y_sb[:])
            return run

        drip = deque()

        def pump():
            if drip:
                drip.popleft()()

        # ---------- attention middle for one (b, pair) ----------
        def attention_pair(b, pr):
            qT, kT = qkTs[(b, pr)]
            v_all = v_alls[b]
            o_allT = o_allTs[b]
            for th in range(NTH):
                esq = [None] * SC
                psos = [ps_o.tile([65, TW], f32, tag="o", name="pso") for _ in range(2)]
                for sc in range(SC + 2):
                    if sc < SC:
                        esq[sc] = p_es.tile([P, 2, TW], bf16, tag="es", name="es")
                        ps = ps_s.tile([P, 2, TW], f32, tag="s", name="ps_s")
                        nc.tensor.matmul(
                            ps[:, 0, :],
                            lhsT=kT[0:64, sc * P : (sc + 1) * P],
                            rhs=qT[0:64, th * TW : (th + 1) * TW],
                            start=True,
                            stop=True,
                        )
                        nc.tensor.matmul(
                            ps[:, 1, :],
                            lhsT=kT[64:128, sc * P : (sc + 1) * P],
                            rhs=qT[64:128, th * TW : (th + 1) * TW],
                            start=True,
                            stop=True,
                            tile_position=(64, 0),
                        )
                        nc.scalar.activation(
                            out=esq[sc][:], in_=ps[:], func=AF.Exp, scale=scale
                        )
                    if sc >= 2:
                        so = sc - 2
                        for h in range(2):
                            nc.tensor.matmul(
                                psos[h][:],
                                lhsT=v_all[:, so, pr, 65 * h : 65 * h + 65],
                                rhs=esq[so][:, h, :],
                                start=(so == 0),
                                stop=(so == SC - 1),
                            )
                    pump()
                # softmax denominators, fully per-head so psos[0] frees early:
                # copy l row to SBUF, DMA to partition 0, invert, broadcast, mul
                l_sb = p_norm.tile([65, 2, TW], f32, tag="l", name="l_sb")
                lg = p_norm.tile([1, 2, TW], f32, tag="lg", name="lg")
                lginv = p_norm.tile([1, 2, TW], f32, tag="lginv", name="lginv")
                linv = p_norm.tile([64, 2, TW], f32, tag="linv", name="linv")
                for h in range(2):
                    nc.vector.tensor_copy(out=l_sb[64:65, h, :], in_=psos[h][64:65, :])
                    nc.sync.dma_start(out=lg[0:1, h, :], in_=l_sb[64:65, h, :])
                    nc.vector.reciprocal_approx_fast(
                        out=lginv[0:1, h, :], in_=lg[0:1, h, :]
                    )
                    nc.gpsimd.partition_broadcast(
                        out_ap=linv[:, h, :],
                        in_ap=lginv[0:1, h, :],
                        channels=64,
                    )
                    if h == 0:
                        nc.vector.tensor_mul(
                            out=o_allT[0:64, pr, th * TW : (th + 1) * TW],
                            in0=psos[h][0:64, :],
                            in1=linv[:, h, :],
                        )
                    else:
                        ot = p_norm.tile([64, TW], bf16, tag="ot", name="ot")
                        nc.vector.tensor_mul(out=ot[:], in0=psos[h][0:64, :], in1=linv[:, h, :])
                        nc.sync.dma_start(
                            out=o_allT[64:128, pr, th * TW : (th + 1) * TW], in_=ot[:]
                        )

        # ================= emission =================
        # HAM warmup: short N=128 matmuls during the initial DMA wait.
        warm = p_norm.tile([P, P], bf16, tag="warm", name="warm")
        nc.vector.memset(warm[:], 0.0)
        wps = ps_m.tile([P, TW], f32, tag="m", name="wps")
        for i in range(32):
            nc.tensor.matmul(
                wps[:, 0:P], lhsT=warm[:], rhs=warm[:], start=(i == 0), stop=(i == 31)
            )

        # prologue: xt(b0), qk(b0, p0) dense; v(b0) drips into pair 0's
        # middle (g0 units first — O(p0, th, sc) needs g0(tci=sc) by iter sc+2)
        load_xt(0)
        prep_vall(0)
        for w in range(2):
            for th in range(NTH):
                qk_unit(0, 0, w, th)()
        for tci in range(TC):
            drip.append(v_group_unit(0, tci, 0))

        for b in range(nb):
            if b + 1 < nb:
                load_xt(b + 1)
                prep_vall(b + 1)
            # y units of the previous batch, spread over this batch's pairs
            ydrip = [[] for _ in range(npair)]
            if b >= 1:
                units = [(tci, j) for tci in range(TC) for j in range(2)]
                alloc = [2, 3, 3, 3, 3, 2]
                k = 0
                for p in range(npair):
                    for _ in range(alloc[p]):
                        ydrip[p].append(units[k]); k += 1
            for pr in range(npair):
                if pr + 1 < npair:
                    for w in range(2):
                        for th in range(NTH):
                            drip.append(qk_unit(b, pr + 1, w, th))
                elif b + 1 < nb:
                    for w in range(2):
                        for th in range(NTH):
                            drip.append(qk_unit(b + 1, 0, w, th))
                if b == 0 and pr == 0:
                    for tci in range(TC):
                        drip.append(v_group_unit(0, tci, 1))
                if b + 1 < nb and 1 <= pr <= 4:
                    for tci in (2 * (pr - 1), 2 * (pr - 1) + 1):
                        for g in range(2):
                            drip.append(v_group_unit(b + 1, tci, g))
                for (tci, j) in ydrip[pr]:
                    drip.append(y_unit(b - 1, tci, j))
                attention_pair(b, pr)
        # drain any leftover drip units, then the last batch's y on ps_s banks
        while drip:
            drip.popleft()()
        for tci in range(TC):
            y_unit_tail(nb - 1, tci)()

    nc.compile()
    return nc


class TileOrExit:
    """Combined TileContext + ExitStack context manager."""

    def __init__(self, nc):
        self.nc = nc
        self.ctx = ExitStack()
        self.tc = tile.TileContext(nc)

    def __enter__(self):
        self.ctx.__enter__()
        self.tc.__enter__()
        return self.tc, self.ctx

    def __exit__(self, *a):
        self.ctx.__exit__(*a)
        return self.tc.__exit__(*a)


def prep_inputs(x, Wq, bq, Wk, bk, Wv, bv, Wp, bp, nb, npair, fp8qk=True):
    """Host-side packing into the DRAM layouts the device kernel expects."""
    P = 128
    t = x.shape[1]
    d = x.shape[2]
    KC = d // P
    KC2 = KC // 2
    dpair = 2 * HS

    def to_bf(a):
        return np.ascontiguousarray(a).astype(BF16)

    xt = np.ascontiguousarray(x.transpose(0, 2, 1)).astype(BF16)  # [B, d, t]

    def pack_qk(W):
        # W: [H, d, HS] -> [P, npair, KC, 2*HS]
        w = W.reshape(npair, 2, KC, P, HS)
        w = w.transpose(3, 0, 2, 1, 4).reshape(P, npair, KC, dpair)
        return np.ascontiguousarray(w)

    if fp8qk:
        wq = (pack_qk(Wq) * W8SCALE).reshape(P, npair, KC2, 2, dpair)
        wk = (pack_qk(Wk) * W8SCALE).reshape(P, npair, KC2, 2, dpair)
        wq = np.clip(wq, -240, 240).astype(F8E4)
        wk = np.clip(wk, -240, 240).astype(F8E4)
        xt8 = np.clip(xt.astype(np.float32), -240, 240).astype(F8E4)
        # bias folded as W8SCALE^2 * b at the exp scale; qT/kT hold 32x values,
        # so the additive bias must be 32x too
        bqk = np.stack(
            [bq.reshape(npair, dpair), bk.reshape(npair, dpair)], axis=-1
        ) * W8SCALE
    else:
        wq = to_bf(pack_qk(Wq))
        wk = to_bf(pack_qk(Wk))
        xt8 = None
        bqk = np.stack(
            [bq.reshape(npair, dpair), bk.reshape(npair, dpair)], axis=-1
        )
    wv = pack_qk(Wv).transpose(0, 2, 1, 3).reshape(P, KC, npair * dpair)
    wv = to_bf(wv)
    wp = to_bf(Wp.reshape(KC, P, d).transpose(1, 0, 2))
    bqk = np.ascontiguousarray(bqk.transpose(1, 0, 2)).astype(np.float32)
    bv_bc = to_bf(np.broadcast_to(bv.reshape(1, npair, dpair), (P, npair, dpair)))
    bp_bc = np.ascontiguousarray(np.broadcast_to(bp.reshape(1, d), (P, d))).astype(
        np.float32
    )

    weights = {
        "wq": wq, "wk": wk, "wv": wv, "wp": wp,
        "bqk": bqk, "bv": bv_bc, "bp": bp_bc,
    }
    n_cores = x.shape[0] // nb
    in_maps = []
    for i in range(n_cores):
        m = dict(weights)
        m["xt"] = np.ascontiguousarray(xt[i * nb : (i + 1) * nb])
        if fp8qk:
            m["xt8"] = np.ascontiguousarray(xt8[i * nb : (i + 1) * nb])
        in_maps.append(m)
    return in_maps


_NC_CACHE = {}
LAST_RESULT = {}
VARIANT = os.environ.get("MHA_VARIANT", "fp8qk")


def kernel(x, Wq, bq, Wk, bk, Wv, bv, Wp, bp, _trace=False):
    x = np.asarray(x, dtype=np.float32)
    Wq, bq = np.asarray(Wq, np.float32), np.asarray(bq, np.float32)
    Wk, bk = np.asarray(Wk, np.float32), np.asarray(bk, np.float32)
    Wv, bv = np.asarray(Wv, np.float32), np.asarray(bv, np.float32)
    Wp, bp = np.asarray(Wp, np.float32), np.asarray(bp, np.float32)

    npair = H // 2
    key = (VARIANT, NB, T_FULL, D_FULL, npair)
    if key not in _NC_CACHE:
        _NC_CACHE[key] = build_mha_nc(NB, T_FULL, D_FULL, npair, variant=VARIANT)
    nc = _NC_CACHE[key]

    in_maps = prep_inputs(
        x, Wq, bq, Wk, bk, Wv, bv, Wp, bp, NB, npair, fp8qk="fp8qk" in VARIANT
    )
    res = run_bass_kernel_spmd(
        nc, in_maps, core_ids=list(range(N_CORES)), trace=_trace
    )
    LAST_RESULT["exec_time_ns"] = res.exec_time_ns
    LAST_RESULT["res"] = res
    outs = [res.results[i]["y"] for i in range(N_CORES)]
    return np.concatenate(outs, axis=0).astype(np.float32)


# revision 9
# speedup vs baseline: 1.3429x; 1.1021x over previous
"""Multi-head attention (B=16, T=1024, D=768, H=12) on 8 TRN2 NeuronCores.

Strategy: pure data parallelism over the batch — each core computes full MHA
for 2 batch elements. No collectives.

v2 changes over the 381us baseline:
  - q/k projections run as fp8e4 DoubleRow matmuls (weights pre-scaled x32 on
    host, bias folded as 32*b, exp scale divided by 32*32). v/O/y stay bf16.
  - All projection work (v-proj, q/k-proj, y-proj) is emitted through a drip
    queue that releases ~1 unit per attention sc-iteration, so the exp-bound
    middle absorbs projection matmuls in its PE slack instead of running them
    as dense PE-only phases.
  - Lean HAM warmup (N=128 matmuls instead of N=512).
  - ps_o gets 3 PSUM banks (th-boundary overlap), drip pool drops to 1.
  - l (softmax denominator) rows leave PSUM by direct DMA (no DVE copy).
  - y(last batch) runs on the ps_s pool banks after attention finishes.
"""

import os
from collections import deque
from contextlib import ExitStack

import numpy as np
import ml_dtypes

import concourse.bacc as bacc
import concourse.bass as bass
import concourse.mybir as mybir
import concourse.tile as tile
from concourse.bass_utils import run_bass_kernel_spmd

BF16 = ml_dtypes.bfloat16
F8E4 = ml_dtypes.float8_e4m3

# Full problem dims
B, T_FULL, D_FULL, H, HS = 16, 1024, 768, 12, 64
N_CORES = 8
NB = B // N_CORES  # batch elements per core
W8SCALE = 32.0     # host pre-scale on Wq/Wk before fp8 cast


def build_mha_nc(nb, t, d, npair, trn_type="TRN2", variant="fp8qk"):
    P = 128
    KC = d // P              # contraction chunks over model dim
    KC2 = KC // 2            # DoubleRow chunk pairs
    SC = t // P              # s (key position) chunks
    NTH = max(1, t // 512)   # output-column groups for S/O matmuls
    TW = t // NTH            # width of each group (<= 512)
    TC = t // P              # t row chunks for v/y
    D2 = d // 2              # y-proj free-dim split (<= 512 fp32 psum)
    dpair = 2 * HS           # 128
    fp8qk = "fp8qk" in variant
    scale = 1.0 / np.sqrt(HS)
    if fp8qk:
        scale /= W8SCALE * W8SCALE

    f32 = mybir.dt.float32
    bf16 = mybir.dt.bfloat16
    f8e4 = mybir.dt.float8e4
    AF = mybir.ActivationFunctionType
    DR = mybir.MatmulPerfMode.DoubleRow

    nc = bacc.Bacc(trn_type, target_bir_lowering=False, debug=False)

    xt_d = nc.dram_tensor("xt", [nb, d, t], bf16, kind="ExternalInput").ap()
    if fp8qk:
        xt8_d = nc.dram_tensor("xt8", [nb, d, t], f8e4, kind="ExternalInput").ap()
        wq_d = nc.dram_tensor("wq", [P, npair, KC2, 2, dpair], f8e4, kind="ExternalInput").ap()
        wk_d = nc.dram_tensor("wk", [P, npair, KC2, 2, dpair], f8e4, kind="ExternalInput").ap()
    else:
        wq_d = nc.dram_tensor("wq", [P, npair, KC, dpair], bf16, kind="ExternalInput").ap()
        wk_d = nc.dram_tensor("wk", [P, npair, KC, dpair], bf16, kind="ExternalInput").ap()
    wv_d = nc.dram_tensor("wv", [P, KC, npair * dpair], bf16, kind="ExternalInput").ap()
    wp_d = nc.dram_tensor("wp", [P, KC, d], bf16, kind="ExternalInput").ap()
    bqk_d = nc.dram_tensor("bqk", [P, npair, 2], f32, kind="ExternalInput").ap()
    bv_d = nc.dram_tensor("bv", [P, npair, dpair], bf16, kind="ExternalInput").ap()
    bp_d = nc.dram_tensor("bp", [P, d], f32, kind="ExternalInput").ap()
    y_d = nc.dram_tensor("y", [nb, t, d], f32, kind="ExternalOutput").ap()

    with TileOrExit(nc) as (tc, ctx):
        # ---- persistent weights ----
        p_w = ctx.enter_context(tc.tile_pool(name="p_w", bufs=1))
        if fp8qk:
            wq_sb = p_w.tile([P, npair, KC2, 2, dpair], f8e4, tag="wq", name="wq_sb")
            wk_sb = p_w.tile([P, npair, KC2, 2, dpair], f8e4, tag="wk", name="wk_sb")
        else:
            wq_sb = p_w.tile([P, npair, KC, dpair], bf16, tag="wq", name="wq_sb")
            wk_sb = p_w.tile([P, npair, KC, dpair], bf16, tag="wk", name="wk_sb")
        wv_sb = p_w.tile([P, KC, npair * dpair], bf16, tag="wv", name="wv_sb")
        wp_sb = p_w.tile([P, KC, d], bf16, tag="wp", name="wp_sb")
        bqk_sb = p_w.tile([P, npair, 2], f32, tag="bqk", name="bqk_sb")
        bv_sb = p_w.tile([P, npair, dpair], bf16, tag="bv", name="bv_sb")
        bp_sb = p_w.tile([P, d], f32, tag="bp", name="bp_sb")
        for pr in range(npair):
            nc.gpsimd.dma_start(wq_sb[:, pr], wq_d[:, pr])
            nc.gpsimd.dma_start(wk_sb[:, pr], wk_d[:, pr])
        nc.gpsimd.dma_start(bqk_sb[:], bqk_d)
        for c in range(KC):
            nc.gpsimd.dma_start(wv_sb[:, c], wv_d[:, c])
        nc.gpsimd.dma_start(bv_sb[:], bv_d)
        nc.gpsimd.dma_start(wp_sb[:], wp_d)
        nc.gpsimd.dma_start(bp_sb[:], bp_d)

        # ---- pools ----
        p_xt = ctx.enter_context(tc.tile_pool(name="p_xt", bufs=2))
        if fp8qk:
            p_xt8 = ctx.enter_context(tc.tile_pool(name="p_xt8", bufs=2))
        p_vall = ctx.enter_context(tc.tile_pool(name="p_vall", bufs=2))
        p_qk = ctx.enter_context(tc.tile_pool(name="p_qk", bufs=3))
        p_es = ctx.enter_context(tc.tile_pool(name="p_es", bufs=6))
        p_oall = ctx.enter_context(tc.tile_pool(name="p_oall", bufs=1))
        p_norm = ctx.enter_context(tc.tile_pool(name="p_norm", bufs=2))
        p_y = ctx.enter_context(tc.tile_pool(name="p_y", bufs=2))
        # PSUM: ps_s 2x2 banks, ps_o 3x1, ps_m 1x1 => 8 banks
        ps_s = ctx.enter_context(tc.tile_pool(name="ps_s", bufs=2, space="PSUM"))
        ps_o = ctx.enter_context(tc.tile_pool(name="ps_o", bufs=3, space="PSUM"))
        ps_m = ctx.enter_context(tc.tile_pool(name="ps_m", bufs=1, space="PSUM"))

        xts = [None] * nb
        xt8s = [None] * nb
        v_alls = [None] * nb
        o_allTs = [None] * nb
        qkTs = {}   # (b, pr) -> (qT, kT)
        y_sbs = {}  # (b, tci) -> y_sb

        def load_xt(b):
            xt = p_xt.tile([P, KC, t], bf16, tag="xt", name="xt_sb")
            xt_src = xt_d[b].rearrange("(c p) t -> p c t", p=P)
            if fp8qk:
                xt8 = p_xt8.tile([P, KC, t], f8e4, tag="xt8", name="xt8_sb")
                xt8_src = xt8_d[b].rearrange("(c p) t -> p c t", p=P)
                for c in range(KC):
                    nc.sync.dma_start(xt8[:, c], xt8_src[:, c])
                xt8s[b] = xt8
            for c in range(KC):
                nc.sync.dma_start(xt[:, c], xt_src[:, c])
            xts[b] = xt

        def prep_vall(b):
            v_alls[b] = p_vall.tile([P, SC, npair, 130], bf16, tag="vall", name="v_all")
            ones_view = v_alls[b].rearrange("p s r (h x) -> p s r h x", h=2)
            nc.gpsimd.memset(ones_view[:, :, :, :, 64:65], 1.0)
            o_allTs[b] = p_oall.tile([P, npair, t], bf16, tag=f"oall{b}", name="o_allT")

        # ---------- drip units ----------
        def v_group_unit(b, tci, g, gn=3):
            def run():
                xt = xts[b]
                psv = ps_m.tile([P, TW], f32, tag="m", name="psv")
                for c in range(KC):
                    nc.tensor.matmul(
                        psv[:, : gn * dpair],
                        lhsT=xt[:, c, tci * P : (tci + 1) * P],
                        rhs=wv_sb[:, c, 3 * g * dpair : (3 * g + gn) * dpair],
                        start=(c == 0),
                        stop=(c == KC - 1),
                    )
                glo = 3 * g
                dst = v_alls[b][:, tci, glo : glo + gn, :].rearrange(
                    "p r (h x) -> p r h x", h=2
                )[:, :, :, 0:64]
                src = psv[:, : gn * dpair].rearrange("p (r h e) -> p r h e", r=gn, h=2)
                bias = bv_sb[:, glo : glo + gn, :].rearrange("p r (h e) -> p r h e", h=2)
                nc.vector.tensor_add(out=dst, in0=src, in1=bias)
            return run

        def qk_unit(b, pr, which, th):
            def run():
                if (b, pr) not in qkTs:
                    qkTs[(b, pr)] = (
                        p_qk.tile([P, t], bf16, tag="qT", name="qT"),
                        p_qk.tile([P, t], bf16, tag="kT", name="kT"),
                    )
                w_sb = wq_sb if which == 0 else wk_sb
                dstT = qkTs[(b, pr)][which]
                psq = ps_m.tile([P, TW], f32, tag="m", name="psq")
                if fp8qk:
                    xt8 = xt8s[b]
                    for i in range(KC2):
                        nc.tensor.matmul(
                            psq[:],
                            lhsT=w_sb[:, pr, i, :, :],
                            rhs=xt8[:, 2 * i : 2 * i + 2, th * TW : (th + 1) * TW],
                            start=(i == 0),
                            stop=(i == KC2 - 1),
                            perf_mode=DR,
                        )
                else:
                    xt = xts[b]
                    for c in range(KC):
                        nc.tensor.matmul(
                            psq[:],
                            lhsT=w_sb[:, pr, c, :],
                            rhs=xt[:, c, th * TW : (th + 1) * TW],
                            start=(c == 0),
                            stop=(c == KC - 1),
                        )
                nc.vector.tensor_scalar_add(
                    out=dstT[:, th * TW : (th + 1) * TW],
                    in0=psq[:],
                    scalar1=bqk_sb[:, pr, which : which + 1],
                )
            return run

        def y_unit(b, tci, j):
            def run():
                if (b, tci) not in y_sbs:
                    y_sbs[(b, tci)] = p_y.tile([P, d], f32, tag="y", name="y_sb")
                y_sb = y_sbs[(b, tci)]
                psy = ps_m.tile([P, TW], f32, tag="m", name="psy")
                for c in range(KC):
                    nc.tensor.matmul(
                        psy[:, 0:D2],
                        lhsT=o_allTs[b][:, c, tci * P : (tci + 1) * P],
                        rhs=wp_sb[:, c, j * D2 : (j + 1) * D2],
                        start=(c == 0),
                        stop=(c == KC - 1),
                    )
                nc.vector.tensor_add(
                    out=y_sb[:, j * D2 : (j + 1) * D2],
                    in0=psy[:, 0:D2],
                    in1=bp_sb[:, j * D2 : (j + 1) * D2],
                )
                if j == 1:
                    nc.sync.dma_start(out=y_d[b, tci * P : (tci + 1) * P, :], in_=y_sb[:])
            return run

        def y_unit_tail(b, tci):
            def run():
                psy = ps_s.tile([P, 2, TW], f32, tag="s", name="ps_s")
                for c in range(KC):
                    for j in range(2):
                        nc.tensor.matmul(
                            psy[:, j, 0:D2],
                            lhsT=o_allTs[b][:, c, tci * P : (tci + 1) * P],
                            rhs=wp_sb[:, c, j * D2 : (j + 1) * D2],
                            start=(c == 0),
                            stop=(c == KC - 1),
                        )
                y_sb = p_y.tile([P, d], f32, tag="y", name="y_sb")
                for j in range(2):
                    nc.vector.tensor_add(
                        out=y_sb[:, j * D2 : (j + 1) * D2],
                        in0=psy[:, j, 0:D2],
                        in1=bp_sb[:, j * D2 : (j + 1) * D2],
                    )
                nc.sync.dma_start(out=y_d[b, tci * P : (tci + 1) * P, :], in_=y_sb[:])
            return run

        drip = deque()

        def pump():
            if drip:
                drip.popleft()()

        # ---------- attention middle for one (b, pair) ----------
        def attention_pair(b, pr):
            qT, kT = qkTs[(b, pr)]
            v_all = v_alls[b]
            o_allT = o_allTs[b]
            for th in range(NTH):
                esq = [None] * SC
                psos = [ps_o.tile([65, TW], f32, tag="o", name="pso") for _ in range(2)]
                for sc in range(SC + 4):
                    if sc < SC:
                        esq[sc] = p_es.tile([P, 2, TW], bf16, tag="es", name="es")
                        ps = ps_s.tile([P, 2, TW], f32, tag="s", name="ps_s")
                        nc.tensor.matmul(
                            ps[:, 0, :],
                            lhsT=kT[0:64, sc * P : (sc + 1) * P],
                            rhs=qT[0:64, th * TW : (th + 1) * TW],
                            start=True,
                            stop=True,
                        )
                        nc.tensor.matmul(
                            ps[:, 1, :],
                            lhsT=kT[64:128, sc * P : (sc + 1) * P],
                            rhs=qT[64:128, th * TW : (th + 1) * TW],
                            start=True,
                            stop=True,
                            tile_position=(64, 0),
                        )
                        nc.scalar.activation(
                            out=esq[sc][:], in_=ps[:], func=AF.Exp, scale=scale
                        )
                    if sc >= 4:
                        so = sc - 4
                        for h in range(2):
                            nc.tensor.matmul(
                                psos[h][:],
                                lhsT=v_all[:, so, pr, 65 * h : 65 * h + 65],
                                rhs=esq[so][:, h, :],
                                start=(so == 0),
                                stop=(so == SC - 1),
                            )
                    pump()
                # softmax denominators, fully per-head so psos[0] frees early:
                # copy l row to SBUF, DMA to partition 0, invert, broadcast, mul
                l_sb = p_norm.tile([65, 2, TW], f32, tag="l", name="l_sb")
                lg = p_norm.tile([1, 2, TW], f32, tag="lg", name="lg")
                lginv = p_norm.tile([1, 2, TW], f32, tag="lginv", name="lginv")
                linv = p_norm.tile([64, 2, TW], f32, tag="linv", name="linv")
                for h in range(2):
                    nc.vector.tensor_copy(out=l_sb[64:65, h, :], in_=psos[h][64:65, :])
                    nc.sync.dma_start(out=lg[0:1, h, :], in_=l_sb[64:65, h, :])
                    nc.vector.reciprocal_approx_fast(
                        out=lginv[0:1, h, :], in_=lg[0:1, h, :]
                    )
                    nc.gpsimd.partition_broadcast(
                        out_ap=linv[:, h, :],
                        in_ap=lginv[0:1, h, :],
                        channels=64,
                    )
                    if h == 0:
                        nc.vector.tensor_mul(
                            out=o_allT[0:64, pr, th * TW : (th + 1) * TW],
                            in0=psos[h][0:64, :],
                            in1=linv[:, h, :],
                        )
                    else:
                        ot = p_norm.tile([64, TW], bf16, tag="ot", name="ot")
                        nc.vector.tensor_mul(out=ot[:], in0=psos[h][0:64, :], in1=linv[:, h, :])
                        nc.sync.dma_start(
                            out=o_allT[64:128, pr, th * TW : (th + 1) * TW], in_=ot[:]
                        )

        # ================= emission =================
        # HAM warmup: short N=128 matmuls during the initial DMA wait.
        warm = p_norm.tile([P, P], bf16, tag="warm", name="warm")
        nc.vector.memset(warm[:], 0.0)
        wps = ps_m.tile([P, TW], f32, tag="m", name="wps")
        for i in range(32):
            nc.tensor.matmul(
                wps[:, 0:P], lhsT=warm[:], rhs=warm[:], start=(i == 0), stop=(i == 31)
            )

        # prologue: xt(b0), qk(b0, p0) dense; v(b0) drips into pair 0's
        # middle (g0 units first — O(p0, th, sc) needs g0(tci=sc) by iter sc+2)
        load_xt(0)
        prep_vall(0)
        for w in range(2):
            for th in range(NTH):
                qk_unit(0, 0, w, th)()
        for tci in range(TC):
            drip.append(v_group_unit(0, tci, 0))

        for b in range(nb):
            if b + 1 < nb:
                load_xt(b + 1)
                prep_vall(b + 1)
            # y units of the previous batch, spread over this batch's pairs
            ydrip = [[] for _ in range(npair)]
            if b >= 1:
                units = [(tci, j) for tci in range(TC) for j in range(2)]
                alloc = [2, 3, 3, 3, 3, 2]
                k = 0
                for p in range(npair):
                    for _ in range(alloc[p]):
                        ydrip[p].append(units[k]); k += 1
            for pr in range(npair):
                if pr + 1 < npair:
                    for w in range(2):
                        for th in range(NTH):
                            drip.append(qk_unit(b, pr + 1, w, th))
                elif b + 1 < nb:
                    for w in range(2):
                        for th in range(NTH):
                            drip.append(qk_unit(b + 1, 0, w, th))
                if b == 0 and pr == 0:
                    for tci in range(TC):
                        drip.append(v_group_unit(0, tci, 1))
                if b + 1 < nb and 1 <= pr <= 4:
                    for tci in (2 * (pr - 1), 2 * (pr - 1) + 1):
                        for g in range(2):
                            drip.append(v_group_unit(b + 1, tci, g))
                for (tci, j) in ydrip[pr]:
                    drip.append(y_unit(b - 1, tci, j))
                attention_pair(b, pr)
        # drain any leftover drip units, then the last batch's y on ps_s banks
        while drip:
            drip.popleft()()
        for tci in range(TC):
            y_unit_tail(nb - 1, tci)()

    nc.compile()
    return nc


class TileOrExit:
    """Combined TileContext + ExitStack context manager."""

    def __init__(self, nc):
        self.nc = nc
        self.ctx = ExitStack()
        self.tc = tile.TileContext(nc)

    def __enter__(self):
        self.ctx.__enter__()
        self.tc.__enter__()
        return self.tc, self.ctx

    def __exit__(self, *a):
        self.ctx.__exit__(*a)
        return self.tc.__exit__(*a)


def prep_inputs(x, Wq, bq, Wk, bk, Wv, bv, Wp, bp, nb, npair, fp8qk=True):
    """Host-side packing into the DRAM layouts the device kernel expects."""
    P = 128
    t = x.shape[1]
    d = x.shape[2]
    KC = d // P
    KC2 = KC // 2
    dpair = 2 * HS

    def to_bf(a):
        return np.ascontiguousarray(a).astype(BF16)

    xt = np.ascontiguousarray(x.transpose(0, 2, 1)).astype(BF16)  # [B, d, t]

    def pack_qk(W):
        # W: [H, d, HS] -> [P, npair, KC, 2*HS]
        w = W.reshape(npair, 2, KC, P, HS)
        w = w.transpose(3, 0, 2, 1, 4).reshape(P, npair, KC, dpair)
        return np.ascontiguousarray(w)

    if fp8qk:
        wq = (pack_qk(Wq) * W8SCALE).reshape(P, npair, KC2, 2, dpair)
        wk = (pack_qk(Wk) * W8SCALE).reshape(P, npair, KC2, 2, dpair)
        wq = np.clip(wq, -240, 240).astype(F8E4)
        wk = np.clip(wk, -240, 240).astype(F8E4)
        xt8 = np.clip(xt.astype(np.float32), -240, 240).astype(F8E4)
        # bias folded as W8SCALE^2 * b at the exp scale; qT/kT hold 32x values,
        # so the additive bias must be 32x too
        bqk = np.stack(
            [bq.reshape(npair, dpair), bk.reshape(npair, dpair)], axis=-1
        ) * W8SCALE
    else:
        wq = to_bf(pack_qk(Wq))
        wk = to_bf(pack_qk(Wk))
        xt8 = None
        bqk = np.stack(
            [bq.reshape(npair, dpair), bk.reshape(npair, dpair)], axis=-1
        )
    wv = pack_qk(Wv).transpose(0, 2, 1, 3).reshape(P, KC, npair * dpair)
    wv = to_bf(wv)
    wp = to_bf(Wp.reshape(KC, P, d).transpose(1, 0, 2))
    bqk = np.ascontiguousarray(bqk.transpose(1, 0, 2)).astype(np.float32)
    bv_bc = to_bf(np.broadcast_to(bv.reshape(1, npair, dpair), (P, npair, dpair)))
    bp_bc = np.ascontiguousarray(np.broadcast_to(bp.reshape(1, d), (P, d))).astype(
        np.float32
    )

    weights = {
        "wq": wq, "wk": wk, "wv": wv, "wp": wp,
        "bqk": bqk, "bv": bv_bc, "bp": bp_bc,
    }
    n_cores = x.shape[0] // nb
    in_maps = []
    for i in range(n_cores):
        m = dict(weights)
        m["xt"] = np.ascontiguousarray(xt[i * nb : (i + 1) * nb])
        if fp8qk:
            m["xt8"] = np.ascontiguousarray(xt8[i * nb : (i + 1) * nb])
        in_maps.append(m)
    return in_maps


_NC_CACHE = {}
LAST_RESULT = {}
VARIANT = os.environ.get("MHA_VARIANT", "fp8qk")


def kernel(x, Wq, bq, Wk, bk, Wv, bv, Wp, bp, _trace=False):
    x = np.asarray(x, dtype=np.float32)
    Wq, bq = np.asarray(Wq, np.float32), np.asarray(bq, np.float32)
    Wk, bk = np.asarray(Wk, np.float32), np.asarray(bk, np.float32)
    Wv, bv = np.asarray(Wv, np.float32), np.asarray(bv, np.float32)
    Wp, bp = np.asarray(Wp, np.float32), np.asarray(bp, np.float32)

    npair = H // 2
    key = (VARIANT, NB, T_FULL, D_FULL, npair)
    if key not in _NC_CACHE:
        _NC_CACHE[key] = build_mha_nc(NB, T_FULL, D_FULL, npair, variant=VARIANT)
    nc = _NC_CACHE[key]

    in_maps = prep_inputs(
        x, Wq, bq, Wk, bk, Wv, bv, Wp, bp, NB, npair, fp8qk="fp8qk" in VARIANT
    )
    res = run_bass_kernel_spmd(
        nc, in_maps, core_ids=list(range(N_CORES)), trace=_trace
    )
    LAST_RESULT["exec_time_ns"] = res.exec_time_ns
    LAST_RESULT["res"] = res
    outs = [res.results[i]["y"] for i in range(N_CORES)]
    return np.concatenate(outs, axis=0).astype(np.float32)
